# revision 2
# baseline (speedup 1.0000x reference)
"""MoE (dense all-expert routing) Trainium2 kernel.

Problem: nn_MixtureOfExperts_56495999811834
  B=4, S=2048, D=1024, H=4096, E=8, K=2  (M = B*S = 8192 tokens)

Reference:
  logits = x @ router_W + router_b; probs = softmax(logits)
  top-2 gate per token (renormalized); h = gelu(x @ W1[e] + b1[e])
  out_e = h @ W2[e] + b2[e]; final = sum_e gate_e * out_e

Sharding: expert-parallel across 8 NeuronCores (one expert per core).
Each core computes gate_e * (gelu(x W1_e + b1_e) W2_e) for its expert over
all 8192 tokens; the host computes the (tiny) router and the final
cross-expert sum (+ the gate-weighted b2 correction, exact since
sum_k w_k = 1 per token).

Device kernel (per core), all matmuls in float32r (full-rate fp32):
  phase 1: hT[h, tok] = gelu(sum_d W1[d,h].T x[d,tok] + b1)   -> DRAM scratch
  phase 2: out[tok, dout] = gate[tok] * sum_h hT[h,tok].T W2[h,dout]
Layouts are pre-shuffled on the host so every DMA is a clean
partition-major pattern.
"""

import numpy as np

# ---- problem constants (hardcoded; kernel.py must be self-contained) ----
B, S, D, H, E, TOPK = 4, 2048, 1024, 4096, 8, 2
M = B * S                    # 8192 tokens
P = 128                      # SBUF partitions
KD = D // P                  # 8  k-subtiles (d contraction)
KH = H // P                  # 32 k-subtiles (h contraction)
TCH = 512                    # phase-1 token chunk
NCH = M // TCH               # 16 chunks
TT = P                       # phase-2 token tile
NTT = M // TT                # 64 token tiles
DCH = 512                    # phase-2 dout chunk
NDC = D // DCH               # 2

TRACE = False                # set by test.py for NTFF profiling
LAST_RESULTS = None          # BassKernelResults of last run (when TRACE)

_STATE = {}


def _ensure_ntff_hook():
    import sys, types
    if "antenv.axon_hooks" not in sys.modules:
        mod = types.ModuleType("antenv.axon_hooks")
        mod._hook = None
        mod.set_axon_ntff_profile_hook = lambda h: setattr(mod, "_hook", h)
        mod.get_axon_ntff_profile_hook = lambda: mod._hook
        sys.modules["antenv.axon_hooks"] = mod
        import antenv
        antenv.axon_hooks = mod
    import antenv.axon_hooks as ah
    if ah.get_axon_ntff_profile_hook() is None:
        try:
            from trn_agent_boot.trn_boot import _ntff_profile_via_ctypes
            ah.set_axon_ntff_profile_hook(
                _ntff_profile_via_ctypes("/opt/axon/libaxon_pjrt.so"))
        except Exception:
            pass


def _build():
    """Build + schedule the Bass program (once per process)."""
    if "nc" in _STATE:
        return _STATE["nc"]

    import concourse.bacc as bacc
    import concourse.mybir as mybir
    import concourse.tile as tile

    f32 = mybir.dt.float32
    f32r = mybir.dt.float32r
    GELU = mybir.ActivationFunctionType.Gelu_apprx_tanh

    nc = bacc.Bacc("TRN2", target_bir_lowering=False, debug=False,
                   num_devices=E)

    # host-preshuffled inputs (see kernel() for the exact shuffles)
    xt_d = nc.dram_tensor("xt", [NCH, P, KD, TCH], f32r, kind="ExternalInput").ap()
    w1_d = nc.dram_tensor("w1", [P, KD, H], f32r, kind="ExternalInput").ap()
    b1_d = nc.dram_tensor("b1", [P, KH], f32, kind="ExternalInput").ap()
    w2_d = nc.dram_tensor("w2", [P, KH, D], f32r, kind="ExternalInput").ap()
    gate_d = nc.dram_tensor("gate", [P, NTT], f32, kind="ExternalInput").ap()
    out_d = nc.dram_tensor("out", [M, D], f32, kind="ExternalOutput").ap()
    # phase-1 -> phase-2 scratch: hT[ht, p, tok]
    ht_d = nc.dram_tensor("ht_scr", [KH, P, M], f32r).ap()

    with tile.TileContext(nc) as tc:
        # ---------------- phase 1: hT = gelu(x @ W1 + b1) ----------------
        with tc.tile_pool(name="w1", bufs=1) as w1_pool, \
             tc.tile_pool(name="b1", bufs=1) as b1_pool, \
             tc.tile_pool(name="xt", bufs=2) as xt_pool, \
             tc.tile_pool(name="hsb", bufs=6) as h_pool, \
             tc.tile_pool(name="ps1", bufs=8, space="PSUM") as ps_pool:

            w1_sb = w1_pool.tile([P, KD, H], f32r)
            for hg in range(8):  # chunked so matmuls can start early
                sl = slice(hg * 512, (hg + 1) * 512)
                nc.sync.dma_start(out=w1_sb[:, :, sl], in_=w1_d[:, :, sl])
            b1_sb = b1_pool.tile([P, KH], f32)
            nc.sync.dma_start(out=b1_sb[:], in_=b1_d[:])

            for c in range(NCH):
                xt_sb = xt_pool.tile([P, KD, TCH], f32r)
                nc.sync.dma_start(out=xt_sb[:], in_=xt_d[c])
                for ht in range(KH):
                    ps = ps_pool.tile([P, TCH], f32)
                    for k in range(KD):
                        nc.tensor.matmul(
                            ps[:],
                            lhsT=w1_sb[:, k, ht * P:(ht + 1) * P],
                            rhs=xt_sb[:, k, :],
                            start=(k == 0), stop=(k == KD - 1),
                        )
                    h_sb = h_pool.tile([P, TCH], f32r)
                    nc.scalar.activation(h_sb[:], ps[:], GELU,
                                         bias=b1_sb[:, ht:ht + 1])
                    nc.sync.dma_start(
                        out=ht_d[ht, :, c * TCH:(c + 1) * TCH], in_=h_sb[:])

        # ------------- phase 2: out = gate * (hT.T @ W2) -----------------
        with tc.tile_pool(name="w2", bufs=1) as w2_pool, \
             tc.tile_pool(name="gate", bufs=1) as g_pool, \
             tc.tile_pool(name="htc", bufs=2) as htc_pool, \
             tc.tile_pool(name="osb", bufs=4) as o_pool, \
             tc.tile_pool(name="ps2", bufs=8, space="PSUM") as ps2_pool:

            w2_sb = w2_pool.tile([P, KH, D], f32r)
            for g in range(8):
                sl = slice(g * 4, (g + 1) * 4)
                nc.sync.dma_start(out=w2_sb[:, sl, :], in_=w2_d[:, sl, :])
            gate_sb = g_pool.tile([P, NTT], f32)
            nc.sync.dma_start(out=gate_sb[:], in_=gate_d[:])

            for tt in range(NTT):
                htc = htc_pool.tile([P, KH, TT], f32r)
                nc.sync.dma_start(
                    out=htc[:],
                    in_=ht_d[:, :, tt * TT:(tt + 1) * TT].rearrange(
                        "h p t -> p h t"))
                for dc in range(NDC):
                    ps = ps2_pool.tile([P, DCH], f32)
                    for ht in range(KH):
                        nc.tensor.matmul(
                            ps[:],
                            lhsT=htc[:, ht, :],
                            rhs=w2_sb[:, ht, dc * DCH:(dc + 1) * DCH],
                            start=(ht == 0), stop=(ht == KH - 1),
                        )
                    o_sb = o_pool.tile([P, DCH], f32)
                    nc.vector.tensor_scalar_mul(o_sb[:], ps[:],
                                                gate_sb[:, tt:tt + 1])
                    nc.sync.dma_start(
                        out=out_d[tt * TT:(tt + 1) * TT,
                                  dc * DCH:(dc + 1) * DCH],
                        in_=o_sb[:])

    nc.compile()
    _STATE["nc"] = nc
    return nc


def _router_gates(x2d, router_W, router_b):
    """Top-2 renormalized softmax gates, [M, E] fp32 (host, exact math)."""
    logits = (x2d @ router_W + router_b).astype(np.float32)
    # top-2 indices; mergesort is stable so ties resolve to the lower
    # index, matching jax.lax.top_k
    order = np.argsort(-logits, axis=1, kind="stable")[:, :TOPK]
    ar = np.arange(M)
    l1 = logits[ar, order[:, 0]]
    l2 = logits[ar, order[:, 1]]
    e2 = np.exp(l2 - l1)
    w1 = 1.0 / (1.0 + e2)
    w2 = e2 / (1.0 + e2)
    gates = np.zeros((M, E), np.float32)
    gates[ar, order[:, 0]] = w1
    gates[ar, order[:, 1]] = w2
    return gates


def kernel(x, router_W, router_b, W1, b1, W2, b2):
    global LAST_RESULTS
    x = np.asarray(x, np.float32)
    router_W = np.asarray(router_W, np.float32)
    router_b = np.asarray(router_b, np.float32)
    W1 = np.asarray(W1, np.float32)
    b1 = np.asarray(b1, np.float32)
    W2 = np.asarray(W2, np.float32)
    b2 = np.asarray(b2, np.float32)

    x2d = np.ascontiguousarray(x.reshape(M, D))
    gates = _router_gates(x2d, router_W, router_b)

    # xt[c, p, k, t] = x[c*TCH + t, k*P + p]
    xt = np.ascontiguousarray(
        x2d.reshape(NCH, TCH, KD, P).transpose(0, 3, 2, 1))

    if TRACE:
        _ensure_ntff_hook()
    nc = _build()
    from concourse.bass_utils import run_bass_kernel_spmd

    in_maps = []
    for e in range(E):
        in_maps.append({
            "xt": xt,
            # w1[p, k, h] = W1[e, k*P+p, h]
            "w1": np.ascontiguousarray(
                W1[e].reshape(KD, P, H).transpose(1, 0, 2)),
            # b1[p, ht] = b1[e, ht*P+p]
            "b1": np.ascontiguousarray(b1[e].reshape(KH, P).T),
            # w2[p, ht, d] = W2[e, ht*P+p, d]
            "w2": np.ascontiguousarray(
                W2[e].reshape(KH, P, D).transpose(1, 0, 2)),
            # gate[p, tt] = gates[tt*P+p, e]
            "gate": np.ascontiguousarray(gates[:, e].reshape(NTT, P).T),
        })

    res = run_bass_kernel_spmd(nc, in_maps, core_ids=list(range(E)),
                               trace=TRACE)
    LAST_RESULTS = res

    final = np.zeros((M, D), np.float32)
    for e in range(E):
        final += res.results[e]["out"]
    # exact b2 correction: sum_e gate_e * b2_e
    final += gates @ b2
    return final.reshape(B, S, D), np.float32(0.0)


# revision 6
# speedup vs baseline: 3.7342x; 3.7342x over previous
"""MoE (top-2 routed) Trainium2 kernel.

Problem: nn_MixtureOfExperts_56495999811834
  B=4, S=2048, D=1024, H=4096, E=8, K=2  (M = B*S = 8192 tokens)

Reference (dense form):
  logits = x @ router_W + router_b; probs = softmax(logits)
  top-2 gate per token (renormalized); h = gelu(x @ W1[e] + b1[e])
  out_e = h @ W2[e] + b2[e]; final = sum_e gate_e * out_e

The gate is zero for all but the top-2 experts of each token, so only
~2/E of the dense expert compute contributes to the output. The host
runs the (tiny) router, gathers each expert's routed tokens (padded to
a 512 multiple), and each of the 8 NeuronCores runs one expert's MLP on
just its ~M*2/E tokens. Results are scattered back and summed on the
host (+ the gate-weighted b2 correction, exact since sum_k w_k = 1).

Per core, the expert's H is processed in two sequential halves; one
half's W1/W2 slices fit in SBUF (64+64 KB/partition, f32), so hidden
activations never touch DRAM. Weight halves swap mid-kernel on a
separate DMA ring, hidden behind compute. All matmuls use float32r
(full-rate fp32, ~1e-4 matmul precision).

Device kernel (per core, per H-half q, per 512-token chunk c):
    hT[h, tok] = gelu(sum_d W1h[d,h].T x[d,tok] + b1h)       (SBUF only)
    per 128-token subtile, 512-wide dout chunk:
      out[q, tok, dout] = gate[tok] * sum_h hT[h,tok].T W2h[h,dout]
Layouts are pre-shuffled on the host so every DMA is a clean
partition-major pattern.
"""

import numpy as np

# ---- problem constants (hardcoded; kernel.py must be self-contained) ----
B, S, D, H, E, TOPK = 4, 2048, 1024, 4096, 8, 2
M = B * S                    # 8192 tokens
P = 128                      # SBUF partitions
KD = D // P                  # 8  k-subtiles (d contraction)
HH = H // 2                  # 2048 hidden per half
KH = HH // P                 # 16 k-subtiles (h contraction per half)
TCH = 512                    # token chunk
NSUB = TCH // P              # 4 token subtiles per chunk
DCH = 512                    # dout chunk
NDC = D // DCH               # 2

TRACE = False                # set by test.py for NTFF profiling
LAST_RESULTS = None          # BassKernelResults of last run (when TRACE)

_STATE = {}


def _ensure_ntff_hook():
    import sys, types
    if "antenv.axon_hooks" not in sys.modules:
        mod = types.ModuleType("antenv.axon_hooks")
        mod._hook = None
        mod.set_axon_ntff_profile_hook = lambda h: setattr(mod, "_hook", h)
        mod.get_axon_ntff_profile_hook = lambda: mod._hook
        sys.modules["antenv.axon_hooks"] = mod
        import antenv
        antenv.axon_hooks = mod
    import antenv.axon_hooks as ah
    if ah.get_axon_ntff_profile_hook() is None:
        try:
            from trn_agent_boot.trn_boot import _ntff_profile_via_ctypes
            ah.set_axon_ntff_profile_hook(
                _ntff_profile_via_ctypes("/opt/axon/libaxon_pjrt.so"))
        except Exception:
            pass


def _build(nch):
    """Build + schedule the Bass program for nch 512-token chunks/core."""
    if nch in _STATE:
        return _STATE[nch]

    import concourse.bacc as bacc
    import concourse.mybir as mybir
    import concourse.tile as tile

    f32 = mybir.dt.float32
    f32r = mybir.dt.float32r
    GELU = mybir.ActivationFunctionType.Gelu_apprx_tanh

    ncap = nch * TCH             # padded tokens per core
    ntt = ncap // P              # 128-token tiles per core

    nc = bacc.Bacc("TRN2", target_bir_lowering=False, debug=False,
                   num_devices=E)

    # host-preshuffled inputs (see kernel() for the exact shuffles)
    # xt[c, p, k, t] = xsel[c*TCH + t, k*P + p]
    xt_d = nc.dram_tensor("xt", [nch, P, KD, TCH], f32r,
                          kind="ExternalInput").ap()
    # w1[q, p, k, h] = W1[e, k*P + p, q*HH + h]
    w1_d = nc.dram_tensor("w1", [2, P, KD, HH], f32r,
                          kind="ExternalInput").ap()
    # b1[p, q*KH + ht] = b1[e, q*HH + ht*P + p]
    b1_d = nc.dram_tensor("b1", [P, 2 * KH], f32, kind="ExternalInput").ap()
    # w2[q, p, ht, d] = W2[e, q*HH + ht*P + p, d]
    w2_d = nc.dram_tensor("w2", [2, P, KH, D], f32r,
                          kind="ExternalInput").ap()
    # gate[p, tt] = gates[sel[tt*P + p], e]   (0 on padding)
    gate_d = nc.dram_tensor("gate", [P, ntt], f32,
                            kind="ExternalInput").ap()
    out_d = nc.dram_tensor("out", [2, ncap, D], f32,
                           kind="ExternalOutput").ap()

    W1PC, W2PC = 8, 8  # DMA pieces per half (1 MB each)

    with tile.TileContext(nc) as tc:
        with tc.tile_pool(name="w1", bufs=1) as w1_pool, \
             tc.tile_pool(name="w2", bufs=1) as w2_pool, \
             tc.tile_pool(name="b1", bufs=1) as b1_pool, \
             tc.tile_pool(name="gate", bufs=1) as g_pool, \
             tc.tile_pool(name="xt", bufs=2) as xt_pool, \
             tc.tile_pool(name="hsb", bufs=KH) as h_pool, \
             tc.tile_pool(name="osb", bufs=4) as o_pool, \
             tc.tile_pool(name="ps", bufs=8, space="PSUM") as ps_pool:

            b1_sb = b1_pool.tile([P, 2 * KH], f32)
            gate_sb = g_pool.tile([P, ntt], f32)
            nc.scalar.dma_start(out=b1_sb[:], in_=b1_d[:])
            nc.scalar.dma_start(out=gate_sb[:], in_=gate_d[:])

            # one half's weights resident at a time; loads go on the ACT
            # HWDGE ring so they never head-of-line-block the xt/out ring
            w1_sb = w1_pool.tile([P, KD, HH], f32r)
            w2_sb = w2_pool.tile([P, KH, D], f32r)

            def load_w1(q):
                for g in range(W1PC):
                    sl = slice(g * (HH // W1PC), (g + 1) * (HH // W1PC))
                    nc.scalar.dma_start(out=w1_sb[:, :, sl],
                                        in_=w1_d[q][:, :, sl])

            def load_w2(q):
                for g in range(W2PC):
                    sl = slice(g * (KH // W2PC), (g + 1) * (KH // W2PC))
                    nc.scalar.dma_start(out=w2_sb[:, sl, :],
                                        in_=w2_d[q][:, sl, :])

            for q in range(2):
                if q == 0:
                    load_w1(0)
                    load_w2(0)
                for c in range(nch):
                    xt_sb = xt_pool.tile([P, KD, TCH], f32r)
                    nc.sync.dma_start(out=xt_sb[:], in_=xt_d[c])

                    # hidden: hT[ht] = gelu(x @ W1h + b1h), kept in SBUF
                    h_tiles = [None] * KH
                    for ht in range(KH):
                        ps = ps_pool.tile([P, TCH], f32, name="ps")
                        for k in range(KD):
                            nc.tensor.matmul(
                                ps[:],
                                lhsT=w1_sb[:, k, ht * P:(ht + 1) * P],
                                rhs=xt_sb[:, k, :],
                                start=(k == 0), stop=(k == KD - 1),
                            )
                        h_sb = h_pool.tile([P, TCH], f32r)
                        nc.scalar.activation(
                            h_sb[:], ps[:], GELU,
                            bias=b1_sb[:, q * KH + ht:q * KH + ht + 1])
                        h_tiles[ht] = h_sb
                    if q == 0 and c == nch - 1:
                        # done reading W1h[0]; stream W1h[1] behind the
                        # remaining phase-2 work
                        load_w1(1)

                    # out[q, tok, dout] = gate * (hT.T @ W2h)
                    for sub in range(NSUB):
                        tt = c * NSUB + sub
                        tsl = slice(sub * P, (sub + 1) * P)
                        for dc in range(NDC):
                            dsl = slice(dc * DCH, (dc + 1) * DCH)
                            ps = ps_pool.tile([P, DCH], f32, name="ps")
                            for ht in range(KH):
                                nc.tensor.matmul(
                                    ps[:],
                                    lhsT=h_tiles[ht][:, tsl],
                                    rhs=w2_sb[:, ht, dsl],
                                    start=(ht == 0), stop=(ht == KH - 1),
                                )
                            o_sb = o_pool.tile([P, DCH], f32)
                            nc.vector.tensor_scalar_mul(
                                o_sb[:], ps[:], gate_sb[:, tt:tt + 1])
                            nc.sync.dma_start(
                                out=out_d[q, tt * P:(tt + 1) * P, dsl],
                                in_=o_sb[:])
                if q == 0:
                    load_w2(1)  # hides behind half-1's first phase-1 chunk

    nc.compile()
    _STATE[nch] = nc
    return nc


def _router_gates(x2d, router_W, router_b):
    """Top-2 renormalized softmax gates, [M, E] fp32 (host, exact math)."""
    logits = (x2d @ router_W + router_b).astype(np.float32)
    # stable sort ties resolve to the lower index, matching jax.lax.top_k
    order = np.argsort(-logits, axis=1, kind="stable")[:, :TOPK]
    ar = np.arange(M)
    l1 = logits[ar, order[:, 0]]
    l2 = logits[ar, order[:, 1]]
    e2 = np.exp(l2 - l1)
    w1 = 1.0 / (1.0 + e2)
    w2 = e2 / (1.0 + e2)
    gates = np.zeros((M, E), np.float32)
    gates[ar, order[:, 0]] = w1
    gates[ar, order[:, 1]] = w2
    return gates


def kernel(x, router_W, router_b, W1, b1, W2, b2):
    global LAST_RESULTS
    x = np.asarray(x, np.float32)
    router_W = np.asarray(router_W, np.float32)
    router_b = np.asarray(router_b, np.float32)
    W1 = np.asarray(W1, np.float32)
    b1 = np.asarray(b1, np.float32)
    W2 = np.asarray(W2, np.float32)
    b2 = np.asarray(b2, np.float32)

    x2d = np.ascontiguousarray(x.reshape(M, D))
    gates = _router_gates(x2d, router_W, router_b)

    sels = [np.nonzero(gates[:, e] > 0.0)[0] for e in range(E)]
    nmax = max(len(s) for s in sels)
    nch = max(1, -(-nmax // TCH))        # chunks of 512, >= max expert load
    ncap = nch * TCH
    ntt = ncap // P

    if TRACE:
        _ensure_ntff_hook()
    nc = _build(nch)
    from concourse.bass_utils import run_bass_kernel_spmd

    in_maps = []
    for e in range(E):
        sel = sels[e]
        xsel = np.zeros((ncap, D), np.float32)
        xsel[:len(sel)] = x2d[sel]
        gsel = np.zeros(ncap, np.float32)
        gsel[:len(sel)] = gates[sel, e]
        w1h = np.stack([
            W1[e, :, :HH].reshape(KD, P, HH).transpose(1, 0, 2),
            W1[e, :, HH:].reshape(KD, P, HH).transpose(1, 0, 2),
        ])
        b1h = np.concatenate([
            b1[e, :HH].reshape(KH, P).T, b1[e, HH:].reshape(KH, P).T,
        ], axis=1)
        w2h = np.stack([
            W2[e, :HH].reshape(KH, P, D).transpose(1, 0, 2),
            W2[e, HH:].reshape(KH, P, D).transpose(1, 0, 2),
        ])
        in_maps.append({
            # xt[c, p, k, t] = xsel[c*TCH + t, k*P + p]
            "xt": np.ascontiguousarray(
                xsel.reshape(nch, TCH, KD, P).transpose(0, 3, 2, 1)),
            "w1": np.ascontiguousarray(w1h),
            "b1": np.ascontiguousarray(b1h),
            "w2": np.ascontiguousarray(w2h),
            "gate": np.ascontiguousarray(gsel.reshape(ntt, P).T),
        })

    res = run_bass_kernel_spmd(nc, in_maps, core_ids=list(range(E)),
                               trace=TRACE)
    LAST_RESULTS = res

    final = np.zeros((M, D), np.float32)
    for e in range(E):
        sel = sels[e]
        o = res.results[e]["out"]
        final[sel] += o[0][:len(sel)] + o[1][:len(sel)]  # sel is unique
    # exact b2 correction: sum_e gate_e * b2_e
    final += gates @ b2
    return final.reshape(B, S, D), np.float32(0.0)


# revision 8
# speedup vs baseline: 4.1856x; 1.1209x over previous
"""MoE (top-2 routed) Trainium2 kernel.

Problem: nn_MixtureOfExperts_56495999811834
  B=4, S=2048, D=1024, H=4096, E=8, K=2  (M = B*S = 8192 tokens)

Reference (dense form):
  logits = x @ router_W + router_b; probs = softmax(logits)
  top-2 gate per token (renormalized); h = gelu(x @ W1[e] + b1[e])
  out_e = h @ W2[e] + b2[e]; final = sum_e gate_e * out_e

The gate is zero for all but the top-2 experts of each token, so only
~2/E of the dense expert compute contributes to the output. The host
runs the (tiny) router, gathers each expert's routed tokens (padded to
a 512 multiple), and each of the 8 NeuronCores runs one expert's MLP on
just its ~M*2/E tokens. Results are scattered back and summed on the
host (+ the gate-weighted b2 correction, exact since sum_k w_k = 1).

Per core, the expert's H is processed in two sequential halves; one
half's W1/W2 slices fit in SBUF (64+64 KB/partition, f32), so hidden
activations never touch DRAM. Weight halves swap mid-kernel on a
separate DMA ring, hidden behind compute. All matmuls use float32r
(full-rate fp32, ~1e-4 matmul precision).

Device kernel (per core, per H-half q, per 512-token chunk c):
    hT[h, tok] = gelu(sum_d W1h[d,h].T x[d,tok] + b1h)       (SBUF only)
    per 128-token subtile, 512-wide dout chunk:
      out[q, tok, dout] = gate[tok] * sum_h hT[h,tok].T W2h[h,dout]
Layouts are pre-shuffled on the host so every DMA is a clean
partition-major pattern.
"""

import numpy as np

# ---- problem constants (hardcoded; kernel.py must be self-contained) ----
B, S, D, H, E, TOPK = 4, 2048, 1024, 4096, 8, 2
M = B * S                    # 8192 tokens
P = 128                      # SBUF partitions
KD = D // P                  # 8  k-subtiles (d contraction)
HH = H // 2                  # 2048 hidden per half
KH = HH // P                 # 16 k-subtiles (h contraction per half)
TCH = 512                    # token chunk
NSUB = TCH // P              # 4 token subtiles per chunk
DCH = 512                    # dout chunk
NDC = D // DCH               # 2

TRACE = False                # set by test.py for NTFF profiling
LAST_RESULTS = None          # BassKernelResults of last run (when TRACE)

_STATE = {}


def _ensure_ntff_hook():
    import sys, types
    if "antenv.axon_hooks" not in sys.modules:
        mod = types.ModuleType("antenv.axon_hooks")
        mod._hook = None
        mod.set_axon_ntff_profile_hook = lambda h: setattr(mod, "_hook", h)
        mod.get_axon_ntff_profile_hook = lambda: mod._hook
        sys.modules["antenv.axon_hooks"] = mod
        import antenv
        antenv.axon_hooks = mod
    import antenv.axon_hooks as ah
    if ah.get_axon_ntff_profile_hook() is None:
        try:
            from trn_agent_boot.trn_boot import _ntff_profile_via_ctypes
            ah.set_axon_ntff_profile_hook(
                _ntff_profile_via_ctypes("/opt/axon/libaxon_pjrt.so"))
        except Exception:
            pass


def _build(chunks):
    """Build + schedule the Bass program; `chunks` = per-chunk token counts
    (multiples of 128, each <= 512)."""
    chunks = tuple(chunks)
    if chunks in _STATE:
        return _STATE[chunks]

    import concourse.bacc as bacc
    import concourse.mybir as mybir
    import concourse.tile as tile

    f32 = mybir.dt.float32
    f32r = mybir.dt.float32r
    GELU = mybir.ActivationFunctionType.Gelu_apprx_tanh

    ncap = sum(chunks)           # padded tokens per core
    ntt = ncap // P              # 128-token tiles per core
    offs = [sum(chunks[:i]) for i in range(len(chunks))]

    nc = bacc.Bacc("TRN2", target_bir_lowering=False, debug=False,
                   num_devices=E)

    # host-preshuffled inputs (see kernel() for the exact shuffles)
    # xt[p, k, t] = xsel[t, k*P + p]
    xt_d = nc.dram_tensor("xt", [P, KD, ncap], f32r,
                          kind="ExternalInput").ap()
    # w1[q, p, k, h] = W1[e, k*P + p, q*HH + h]
    w1_d = nc.dram_tensor("w1", [2, P, KD, HH], f32r,
                          kind="ExternalInput").ap()
    # b1[p, q*KH + ht] = b1[e, q*HH + ht*P + p]
    b1_d = nc.dram_tensor("b1", [P, 2 * KH], f32, kind="ExternalInput").ap()
    # w2[q, p, ht, d] = W2[e, q*HH + ht*P + p, d]
    w2_d = nc.dram_tensor("w2", [2, P, KH, D], f32r,
                          kind="ExternalInput").ap()
    # gate[p, tt] = gates[sel[tt*P + p], e]   (0 on padding)
    gate_d = nc.dram_tensor("gate", [P, ntt], f32,
                            kind="ExternalInput").ap()
    out_d = nc.dram_tensor("out", [2, ncap, D], f32,
                           kind="ExternalOutput").ap()

    W1PC, W2PC = 8, 8  # DMA pieces per half (1 MB each)

    with tile.TileContext(nc) as tc:
        with tc.tile_pool(name="w1", bufs=1) as w1_pool, \
             tc.tile_pool(name="w2", bufs=1) as w2_pool, \
             tc.tile_pool(name="b1", bufs=1) as b1_pool, \
             tc.tile_pool(name="gate", bufs=1) as g_pool, \
             tc.tile_pool(name="xt", bufs=2) as xt_pool, \
             tc.tile_pool(name="hsb", bufs=KH) as h_pool, \
             tc.tile_pool(name="osb", bufs=4) as o_pool, \
             tc.tile_pool(name="ps", bufs=8, space="PSUM") as ps_pool:

            b1_sb = b1_pool.tile([P, 2 * KH], f32)
            gate_sb = g_pool.tile([P, ntt], f32)
            nc.scalar.dma_start(out=b1_sb[:], in_=b1_d[:])
            nc.scalar.dma_start(out=gate_sb[:], in_=gate_d[:])

            # one half's weights resident at a time; loads go on the ACT
            # HWDGE ring so they never head-of-line-block the xt/out ring
            w1_sb = w1_pool.tile([P, KD, HH], f32r)
            w2_sb = w2_pool.tile([P, KH, D], f32r)

            def load_w1(q):
                for g in range(W1PC):
                    sl = slice(g * (HH // W1PC), (g + 1) * (HH // W1PC))
                    nc.scalar.dma_start(out=w1_sb[:, :, sl],
                                        in_=w1_d[q][:, :, sl])

            def load_w2(q):
                for g in range(W2PC):
                    sl = slice(g * (KH // W2PC), (g + 1) * (KH // W2PC))
                    nc.scalar.dma_start(out=w2_sb[:, sl, :],
                                        in_=w2_d[q][:, sl, :])

            for q in range(2):
                if q == 0:
                    load_w1(0)
                    load_w2(0)
                for c, tch in enumerate(chunks):
                    tok0 = offs[c]
                    xt_sb = xt_pool.tile([P, KD, tch], f32r, name="xt")
                    nc.sync.dma_start(
                        out=xt_sb[:], in_=xt_d[:, :, tok0:tok0 + tch])

                    # hidden: hT[ht] = gelu(x @ W1h + b1h), kept in SBUF
                    h_tiles = [None] * KH
                    for ht in range(KH):
                        ps = ps_pool.tile([P, tch], f32, name="ps")
                        for k in range(KD):
                            nc.tensor.matmul(
                                ps[:],
                                lhsT=w1_sb[:, k, ht * P:(ht + 1) * P],
                                rhs=xt_sb[:, k, :],
                                start=(k == 0), stop=(k == KD - 1),
                            )
                        h_sb = h_pool.tile([P, tch], f32r, name="h_sb")
                        nc.scalar.activation(
                            h_sb[:], ps[:], GELU,
                            bias=b1_sb[:, q * KH + ht:q * KH + ht + 1])
                        h_tiles[ht] = h_sb
                    if q == 0 and c == len(chunks) - 1:
                        # done reading W1h[0]; stream W1h[1] behind the
                        # remaining phase-2 work
                        load_w1(1)

                    # out[q, tok, dout] = gate * (hT.T @ W2h)
                    for sub in range(tch // P):
                        tt = tok0 // P + sub
                        tsl = slice(sub * P, (sub + 1) * P)
                        for dc in range(NDC):
                            dsl = slice(dc * DCH, (dc + 1) * DCH)
                            ps = ps_pool.tile([P, DCH], f32, name="ps")
                            for ht in range(KH):
                                nc.tensor.matmul(
                                    ps[:],
                                    lhsT=h_tiles[ht][:, tsl],
                                    rhs=w2_sb[:, ht, dsl],
                                    start=(ht == 0), stop=(ht == KH - 1),
                                )
                            o_sb = o_pool.tile([P, DCH], f32)
                            nc.vector.tensor_scalar_mul(
                                o_sb[:], ps[:], gate_sb[:, tt:tt + 1])
                            nc.sync.dma_start(
                                out=out_d[q, tt * P:(tt + 1) * P, dsl],
                                in_=o_sb[:])
                if q == 0:
                    load_w2(1)  # hides behind half-1's first phase-1 chunk

    nc.compile()
    _STATE[chunks] = nc
    return nc


def _router_gates(x2d, router_W, router_b):
    """Top-2 renormalized softmax gates, [M, E] fp32 (host, exact math)."""
    logits = (x2d @ router_W + router_b).astype(np.float32)
    # stable sort ties resolve to the lower index, matching jax.lax.top_k
    order = np.argsort(-logits, axis=1, kind="stable")[:, :TOPK]
    ar = np.arange(M)
    l1 = logits[ar, order[:, 0]]
    l2 = logits[ar, order[:, 1]]
    e2 = np.exp(l2 - l1)
    w1 = 1.0 / (1.0 + e2)
    w2 = e2 / (1.0 + e2)
    gates = np.zeros((M, E), np.float32)
    gates[ar, order[:, 0]] = w1
    gates[ar, order[:, 1]] = w2
    return gates


def kernel(x, router_W, router_b, W1, b1, W2, b2):
    global LAST_RESULTS
    x = np.asarray(x, np.float32)
    router_W = np.asarray(router_W, np.float32)
    router_b = np.asarray(router_b, np.float32)
    W1 = np.asarray(W1, np.float32)
    b1 = np.asarray(b1, np.float32)
    W2 = np.asarray(W2, np.float32)
    b2 = np.asarray(b2, np.float32)

    x2d = np.ascontiguousarray(x.reshape(M, D))
    gates = _router_gates(x2d, router_W, router_b)

    sels = [np.nonzero(gates[:, e] > 0.0)[0] for e in range(E)]
    nmax = max(len(s) for s in sels)
    ncap = max(P, -(-nmax // P) * P)     # pad to 128 granularity
    chunks = [TCH] * (ncap // TCH)
    if ncap % TCH:
        chunks.append(ncap % TCH)
    ntt = ncap // P

    if TRACE:
        _ensure_ntff_hook()
    nc = _build(chunks)
    from concourse.bass_utils import run_bass_kernel_spmd

    in_maps = []
    for e in range(E):
        sel = sels[e]
        xsel = np.zeros((ncap, D), np.float32)
        xsel[:len(sel)] = x2d[sel]
        gsel = np.zeros(ncap, np.float32)
        gsel[:len(sel)] = gates[sel, e]
        w1h = np.stack([
            W1[e, :, :HH].reshape(KD, P, HH).transpose(1, 0, 2),
            W1[e, :, HH:].reshape(KD, P, HH).transpose(1, 0, 2),
        ])
        b1h = np.concatenate([
            b1[e, :HH].reshape(KH, P).T, b1[e, HH:].reshape(KH, P).T,
        ], axis=1)
        w2h = np.stack([
            W2[e, :HH].reshape(KH, P, D).transpose(1, 0, 2),
            W2[e, HH:].reshape(KH, P, D).transpose(1, 0, 2),
        ])
        in_maps.append({
            # xt[p, k, t] = xsel[t, k*P + p]
            "xt": np.ascontiguousarray(
                xsel.reshape(ncap, KD, P).transpose(2, 1, 0)),
            "w1": np.ascontiguousarray(w1h),
            "b1": np.ascontiguousarray(b1h),
            "w2": np.ascontiguousarray(w2h),
            "gate": np.ascontiguousarray(gsel.reshape(ntt, P).T),
        })

    res = run_bass_kernel_spmd(nc, in_maps, core_ids=list(range(E)),
                               trace=TRACE)
    LAST_RESULTS = res

    final = np.zeros((M, D), np.float32)
    for e in range(E):
        sel = sels[e]
        o = res.results[e]["out"]
        final[sel] += o[0][:len(sel)] + o[1][:len(sel)]  # sel is unique
    # exact b2 correction: sum_e gate_e * b2_e
    final += gates @ b2
    return final.reshape(B, S, D), np.float32(0.0)
